# revision 6
# baseline (speedup 1.0000x reference)
"""Trainium2 Bass kernel for EnhancedMultiHeadAttention (B=2, S=2048, DM=1024, H=16).

Sharding v2: 8 NeuronCores = 2 batches x 4 query-row blocks of 512 rows.
Each core projects K/V only for its OWN 512 rows (v1 computed them 4x
redundantly), then two AllGathers (K in f32, V in bf16) over the 4 cores of
each batch rebuild the full [1024,2048] K^T / [2048,1024] V in DRAM. The
program is uniform across cores (SPMD): attention consumes all 16 t-tiles
from the gathered tensors, so no per-core indices appear in the
instruction stream.

The output and gate projections are folded into the attention loop: the
host precomputes Wgo = Wg @ Wo and bgo = Wg @ bo + bg, and as each head
pair's context finishes, 8+8 matmuls accumulate its contribution to
out^T and gate_z^T in SBUF (DVE adds). This removes the serial epilogue
matmul chain entirely; the epilogue is elementwise-only in transposed
[DM, 512] layout (sigmoid, gate mix, layernorm with cross-partition
stats via a ones-matmul), and y^T is transposed on the host.

Precision: fp32r matmuls everywhere except attn @ v in bf16 (PSUM
column-packing of the head pair), as in v1. softmax(attention_weights)
is folded into Wv/bv on the host.
"""
import math
import os
import sys

import numpy as np

for _p in ("/opt/trn_rl_repo", "/opt/pypackages"):
    if _p not in sys.path:
        sys.path.append(_p)

import concourse.bass as bass
import concourse.mybir as mybir
import concourse.tile as tile
from concourse import bacc
from concourse.bass_utils import run_bass_kernel_spmd

F32R = mybir.dt.float32r
F32 = mybir.dt.float32
BF16 = mybir.dt.bfloat16
AF = mybir.ActivationFunctionType
ALU = mybir.AluOpType

B, S, DM, H = 2, 2048, 1024, 16
HD = DM // H                  # 64
SQ = 512                      # query rows per core
NP = 128                      # partitions
KC = DM // NP                 # 8 contraction chunks
NT = S // NP                  # 16 key/value tiles
NPAIR = H // 2                # 8 head pairs
NSLOT = 4                     # gather slots (cores per batch)
N512 = 512
SCALE = 1.0 / math.sqrt(HD)
EPS = 1e-5
GROUPS = [[0, 1, 2, 3], [4, 5, 6, 7]]

_CACHE = {}
_TRACE = [False]
_LAST_RESULT = [None]


def _bcast(ap_1d, p=NP):
    return bass.AP(tensor=ap_1d.tensor, offset=ap_1d.offset,
                   ap=[[0, p]] + list(ap_1d.ap))


def _build():
    nc = bacc.Bacc("TRN2", target_bir_lowering=False, debug=False,
                   num_devices=8)

    xT_d = nc.dram_tensor("xT", [DM, SQ], F32R, kind="ExternalInput").ap()
    wkT_d = nc.dram_tensor("wkT", [DM, DM], F32R, kind="ExternalInput").ap()
    wvT_d = nc.dram_tensor("wvT", [DM, DM], F32R, kind="ExternalInput").ap()
    wqT_d = nc.dram_tensor("wqT", [DM, DM], F32R, kind="ExternalInput").ap()
    woT_d = nc.dram_tensor("woT", [DM, DM], F32R, kind="ExternalInput").ap()
    wgoT_d = nc.dram_tensor("wgoT", [DM, DM], F32R, kind="ExternalInput").ap()
    bq_d = nc.dram_tensor("bq", [DM], F32, kind="ExternalInput").ap()
    bk_d = nc.dram_tensor("bk", [DM], F32, kind="ExternalInput").ap()
    bv_d = nc.dram_tensor("bv", [DM], F32, kind="ExternalInput").ap()
    bo_d = nc.dram_tensor("bo", [DM], F32, kind="ExternalInput").ap()
    bgo_d = nc.dram_tensor("bgo", [DM], F32, kind="ExternalInput").ap()
    gam_d = nc.dram_tensor("gam", [DM], F32, kind="ExternalInput").ap()
    bet_d = nc.dram_tensor("bet", [DM], F32, kind="ExternalInput").ap()
    y_d = nc.dram_tensor("y", [DM, SQ], F32, kind="ExternalOutput").ap()

    xT_v = xT_d.rearrange("(c p) s -> p c s", p=NP)
    wk_v = wkT_d.rearrange("(c p) d -> p c d", p=NP)
    wv_v = wvT_d.rearrange("(c p) d -> p c d", p=NP)
    wq_v = wqT_d.rearrange("(c p) d -> p c d", p=NP)
    wo_v = woT_d.rearrange("(c p) d -> p c d", p=NP)
    wgo_v = wgoT_d.rearrange("(c p) d -> p c d", p=NP)
    y_v = y_d.rearrange("(c p) s -> p c s", p=NP)

    with tile.TileContext(nc) as tc:
        with tc.tile_pool(name="pers", bufs=1) as pers, \
             tc.tile_pool(name="acc", bufs=1) as acc, \
             tc.tile_pool(name="dram", bufs=1, space="DRAM") as dram:
            # per-partition bias/affine scalars ([128, 8]: chunk c, part p
            # maps to feature c*128+p)
            bq_sb = pers.tile([NP, KC], F32)
            bk_sb = pers.tile([NP, KC], F32)
            bo_sb = pers.tile([NP, KC], F32)
            bgo_sb = pers.tile([NP, KC], F32)
            gam_sb = pers.tile([NP, KC], F32)
            bet_sb = pers.tile([NP, KC], F32)
            nc.sync.dma_start(out=bq_sb, in_=bq_d.rearrange("(c p) -> p c", p=NP))
            nc.sync.dma_start(out=bk_sb, in_=bk_d.rearrange("(c p) -> p c", p=NP))
            nc.sync.dma_start(out=bo_sb, in_=bo_d.rearrange("(c p) -> p c", p=NP))
            nc.sync.dma_start(out=bgo_sb, in_=bgo_d.rearrange("(c p) -> p c", p=NP))
            nc.sync.dma_start(out=gam_sb, in_=gam_d.rearrange("(c p) -> p c", p=NP))
            nc.sync.dma_start(out=bet_sb, in_=bet_d.rearrange("(c p) -> p c", p=NP))
            bv_bc = pers.tile([NP, DM], F32)
            nc.sync.dma_start(out=bv_bc, in_=_bcast(bv_d))
            eps_sb = pers.tile([NP, 1], F32)
            nc.vector.memset(eps_sb, EPS)
            ones_f32 = pers.tile([NP, NP], F32)
            nc.vector.memset(ones_f32, 1.0)
            ones_sb = pers.tile([NP, NP], F32R)
            nc.gpsimd.dma_start(out=ones_sb, in_=ones_f32)

            # persistent across phases
            xT_sb = acc.tile([NP, KC, SQ], F32R)      # own x^T (also residual)
            outT_acc = acc.tile([NP, KC, SQ], F32R)   # out^T accumulator
            gateT_acc = acc.tile([NP, KC, SQ], F32R)  # gate logits^T accumulator

            # DRAM bounce buffers for the collectives
            kin_dr = dram.tile([NP, KC, SQ], F32R)
            kout_dr = dram.tile([NSLOT, NP, KC, SQ], F32R)
            vin_dr = dram.tile([NP, NSLOT, DM], BF16)
            vout_dr = dram.tile([NSLOT, NP, NSLOT, DM], BF16)

            with tc.tile_pool(name="qres", bufs=1) as qres, \
                 tc.tile_pool(name="kvsh", bufs=1) as kvsh, \
                 tc.tile_pool(name="vqp", bufs=1) as vqp, \
                 tc.tile_pool(name="wsl", bufs=2) as wsl, \
                 tc.tile_pool(name="wop", bufs=4) as wop, \
                 tc.tile_pool(name="kpp", bufs=2) as kpp, \
                 tc.tile_pool(name="ctxp", bufs=2) as ctxp, \
                 tc.tile_pool(name="attp", bufs=3) as attp, \
                 tc.tile_pool(name="pp", bufs=2, space="PSUM") as pp, \
                 tc.tile_pool(name="scop", bufs=2, space="PSUM") as scop, \
                 tc.tile_pool(name="cxp", bufs=2, space="PSUM") as cxp:
                qT_sb = qres.tile([NP, KC, SQ], F32R)
                ksh_sb = kvsh.tile([NP, KC, SQ], F32R)    # own K^T shard
                vsh_sb = kvsh.tile([NP, NSLOT, DM], BF16)  # own V rows shard
                vq_sb = vqp.tile([NP, NT, DM], BF16)       # full gathered V

                # ---- phase 1: own-shard projections + collectives --------
                for kc in range(KC):
                    nc.sync.dma_start(out=xT_sb[:, kc, :], in_=xT_v[:, kc, :])

                # K^T shard: ksh[d, t] for all d, own t
                for dt in range(KC):
                    wk_sl = wsl.tile([NP, KC, NP], F32R, tag="w", name="wk_sl")
                    nc.sync.dma_start(out=wk_sl,
                                      in_=wk_v[:, :, dt * NP:(dt + 1) * NP])
                    ps_t = pp.tile([NP, SQ], F32, tag="pj", name="ps_k")
                    for kc in range(KC):
                        nc.tensor.matmul(ps_t, wk_sl[:, kc, :], xT_sb[:, kc, :],
                                         start=(kc == 0), stop=(kc == KC - 1))
                    nc.vector.tensor_scalar_add(ksh_sb[:, dt, :], ps_t,
                                                bk_sb[:, dt:dt + 1])
                nc.gpsimd.dma_start(out=kin_dr[:], in_=ksh_sb[:])
                nc.gpsimd.collective_compute(
                    "AllGather", mybir.AluOpType.bypass,
                    replica_groups=GROUPS,
                    ins=[kin_dr.opt()], outs=[kout_dr.opt()])

                # V shard: vsh[t, d] for own t, all d (bias + head_w folded)
                for h in range(2):
                    wv_sl = wsl.tile([NP, KC, N512], F32R, tag="wv", name="wv_sl")
                    nc.sync.dma_start(out=wv_sl,
                                      in_=wv_v[:, :, h * N512:(h + 1) * N512])
                    for tt in range(NSLOT):
                        ps_t = pp.tile([NP, N512], F32, tag="pj", name="ps_v")
                        for kc in range(KC):
                            nc.tensor.matmul(
                                ps_t, xT_sb[:, kc, tt * NP:(tt + 1) * NP],
                                wv_sl[:, kc, :],
                                start=(kc == 0), stop=(kc == KC - 1))
                        nc.vector.tensor_add(
                            vsh_sb[:, tt, h * N512:(h + 1) * N512], ps_t,
                            bv_bc[:, h * N512:(h + 1) * N512])
                nc.gpsimd.dma_start(out=vin_dr[:], in_=vsh_sb[:])
                nc.gpsimd.collective_compute(
                    "AllGather", mybir.AluOpType.bypass,
                    replica_groups=GROUPS,
                    ins=[vin_dr.opt()], outs=[vout_dr.opt()])

                # Q^T: [DM, 512]
                for dt in range(KC):
                    wq_sl = wsl.tile([NP, KC, NP], F32R, tag="w", name="wq_sl")
                    nc.sync.dma_start(out=wq_sl,
                                      in_=wq_v[:, :, dt * NP:(dt + 1) * NP])
                    ps_q = pp.tile([NP, SQ], F32, tag="pj", name="ps_q")
                    for kc in range(KC):
                        nc.tensor.matmul(ps_q, wq_sl[:, kc, :], xT_sb[:, kc, :],
                                         start=(kc == 0), stop=(kc == KC - 1))
                    nc.vector.tensor_scalar_add(qT_sb[:, dt, :], ps_q,
                                                bq_sb[:, dt:dt + 1])

                # gathered V -> SBUF (16 t-tiles of [128, 1024] bf16)
                for r in range(NSLOT):
                    nc.gpsimd.dma_start(
                        out=vq_sb[:, r * NSLOT:(r + 1) * NSLOT, :],
                        in_=vout_dr[r, :, :, :])

                # ---- main loop: attention + incremental out/gate ---------
                for p in range(NPAIR):
                    kp = kpp.tile([NP, NT * NP], F32R, tag="kp", name="kp")
                    for r in range(NSLOT):
                        nc.sync.dma_start(
                            out=kp[:, r * N512:(r + 1) * N512],
                            in_=kout_dr[r, :, p, :])
                    wo_sl = wop.tile([NP, DM], F32R, tag="wo", name="wo_sl")
                    nc.sync.dma_start(out=wo_sl, in_=wo_v[:, p, :])
                    wgo_sl = wop.tile([NP, DM], F32R, tag="wo", name="wgo_sl")
                    nc.sync.dma_start(out=wgo_sl, in_=wgo_v[:, p, :])

                    ctx_ps = cxp.tile([NP, SQ], F32, tag="cx", name="ctx_ps")
                    c0 = p * NP
                    for t in range(NT):
                        sco = scop.tile([NP, 2 * SQ], F32, tag="sc", name="sco")
                        nc.tensor.matmul(sco[:, 0:SQ],
                                         kp[0:64, t * NP:(t + 1) * NP],
                                         qT_sb[0:64, p, :],
                                         start=True, stop=True,
                                         tile_position=(0, 0))
                        nc.tensor.matmul(sco[:, SQ:2 * SQ],
                                         kp[64:128, t * NP:(t + 1) * NP],
                                         qT_sb[64:128, p, :],
                                         start=True, stop=True,
                                         tile_position=(64, 0))
                        att_t = attp.tile([NP, 2 * SQ], BF16, tag="at",
                                          name="att_t")
                        nc.scalar.activation(out=att_t, in_=sco, func=AF.Gelu,
                                             scale=SCALE)
                        nc.tensor.matmul(ctx_ps[0:64, :],
                                         vq_sb[:, t, c0:c0 + 64],
                                         att_t[:, 0:SQ],
                                         start=(t == 0), stop=(t == NT - 1),
                                         tile_position=(0, 0))
                        nc.tensor.matmul(ctx_ps[64:128, :],
                                         vq_sb[:, t, c0 + 64:c0 + NP],
                                         att_t[:, SQ:2 * SQ],
                                         start=(t == 0), stop=(t == NT - 1),
                                         tile_position=(0, 64))
                    ctx_sb = ctxp.tile([NP, SQ], F32R, tag="cs", name="ctx_sb")
                    nc.vector.tensor_copy(ctx_sb, ctx_ps)

                    # fold pair p into out^T and gate^T
                    for dt in range(KC):
                        ps_o = pp.tile([NP, SQ], F32, tag="pj", name="ps_o")
                        nc.tensor.matmul(ps_o, wo_sl[:, dt * NP:(dt + 1) * NP],
                                         ctx_sb, start=True, stop=True)
                        if p == 0:
                            nc.vector.tensor_scalar_add(
                                outT_acc[:, dt, :], ps_o, bo_sb[:, dt:dt + 1])
                        else:
                            nc.vector.tensor_add(
                                outT_acc[:, dt, :], ps_o, outT_acc[:, dt, :])
                        ps_g = pp.tile([NP, SQ], F32, tag="pj", name="ps_g")
                        nc.tensor.matmul(ps_g, wgo_sl[:, dt * NP:(dt + 1) * NP],
                                         ctx_sb, start=True, stop=True)
                        if p == 0:
                            nc.vector.tensor_scalar_add(
                                gateT_acc[:, dt, :], ps_g, bgo_sb[:, dt:dt + 1])
                        else:
                            nc.vector.tensor_add(
                                gateT_acc[:, dt, :], ps_g, gateT_acc[:, dt, :])

            # ------------- epilogue: all in transposed [DM, 512] ----------
            with tc.tile_pool(name="ep", bufs=2) as ep, \
                 tc.tile_pool(name="epo", bufs=2) as epo, \
                 tc.tile_pool(name="lns", bufs=1) as lns, \
                 tc.tile_pool(name="spp", bufs=2, space="PSUM") as spp:
                # gate = sigmoid(z)  (in place)
                for g in range(4):
                    nc.scalar.activation(out=gateT_acc[:, 2 * g:2 * g + 2, :],
                                         in_=gateT_acc[:, 2 * g:2 * g + 2, :],
                                         func=AF.Sigmoid)
                musum_ps = spp.tile([NP, SQ], F32, tag="ms", name="musum")
                sqsum_ps = spp.tile([NP, SQ], F32, tag="ms", name="sqsum")
                for dt in range(KC):
                    # ypre = gate*(out - x) + 2x  (overwrites outT_acc)
                    t1 = ep.tile([NP, SQ], F32, tag="t1", name="t1")
                    nc.vector.tensor_sub(t1, outT_acc[:, dt, :], xT_sb[:, dt, :])
                    nc.vector.tensor_mul(t1, t1, gateT_acc[:, dt, :])
                    nc.vector.scalar_tensor_tensor(
                        out=outT_acc[:, dt, :], in0=xT_sb[:, dt, :], scalar=2.0,
                        in1=t1, op0=ALU.mult, op1=ALU.add)
                    sq = ep.tile([NP, SQ], F32R, tag="sq", name="sq")
                    nc.vector.tensor_mul(sq, outT_acc[:, dt, :],
                                         outT_acc[:, dt, :])
                    nc.tensor.matmul(musum_ps, ones_sb, outT_acc[:, dt, :],
                                     start=(dt == 0), stop=(dt == KC - 1))
                    nc.tensor.matmul(sqsum_ps, ones_sb, sq,
                                     start=(dt == 0), stop=(dt == KC - 1))
                mu_sb = lns.tile([NP, SQ], F32)
                m2_sb = lns.tile([NP, SQ], F32)
                var_sb = lns.tile([NP, SQ], F32)
                rstd_sb = lns.tile([NP, SQ], F32)
                nc.vector.tensor_scalar_mul(mu_sb, musum_ps, 1.0 / DM)
                nc.vector.tensor_scalar_mul(m2_sb, sqsum_ps, 1.0 / DM)
                nc.vector.tensor_mul(var_sb, mu_sb, mu_sb)
                nc.vector.tensor_sub(var_sb, m2_sb, var_sb)
                nc.scalar.activation(out=var_sb, in_=var_sb, func=AF.Sqrt,
                                     bias=eps_sb)
                nc.vector.reciprocal(rstd_sb, var_sb)
                for dt in range(KC):
                    yt = epo.tile([NP, SQ], F32, tag="y", name="yt")
                    nc.vector.tensor_sub(yt, outT_acc[:, dt, :], mu_sb)
                    nc.vector.tensor_mul(yt, yt, rstd_sb)
                    nc.vector.tensor_scalar(
                        out=yt, in0=yt,
                        scalar1=gam_sb[:, dt:dt + 1],
                        scalar2=bet_sb[:, dt:dt + 1],
                        op0=ALU.mult, op1=ALU.add)
                    nc.sync.dma_start(out=y_v[:, dt, :], in_=yt)

    nc.compile()
    return nc


def kernel(x, Wq, bq, Wk, bk, Wv, bv, Wo, bo, Wg, bg, attention_weights,
           ln_gamma, ln_beta):
    x = np.asarray(x, dtype=np.float32)
    f32 = lambda a: np.ascontiguousarray(np.asarray(a, dtype=np.float32))
    Wq, Wk, Wv, Wo, Wg = map(f32, (Wq, Wk, Wv, Wo, Wg))
    bq, bk, bv, bo, bg = map(f32, (bq, bk, bv, bo, bg))
    aw, gam, bet = map(f32, (attention_weights, ln_gamma, ln_beta))

    if "nc" not in _CACHE:
        _CACHE["nc"] = _build()
    nc = _CACHE["nc"]

    # fold softmax(attention_weights) into Wv / bv
    e = np.exp(aw - aw.max())
    head_w = (e / e.sum()).astype(np.float32)
    hw_exp = np.repeat(head_w, HD)              # [DM]
    Wv_s = Wv * hw_exp[:, None]
    bv_s = bv * hw_exp

    # fold gate projection through the output projection
    Wgo = (Wg.astype(np.float64) @ Wo.astype(np.float64)).astype(np.float32)
    bgo = (Wg.astype(np.float64) @ bo.astype(np.float64)).astype(np.float32) + bg

    wqT = np.ascontiguousarray(Wq.T)
    wkT = np.ascontiguousarray(Wk.T)
    wvT = np.ascontiguousarray(Wv_s.T)
    woT = np.ascontiguousarray(Wo.T)
    wgoT = np.ascontiguousarray(Wgo.T)

    in_maps = []
    for c in range(8):
        b, blk = divmod(c, 4)
        r0 = blk * SQ
        in_maps.append({
            "xT": np.ascontiguousarray(x[b, r0:r0 + SQ].T),
            "wkT": wkT, "wvT": wvT, "wqT": wqT, "woT": woT, "wgoT": wgoT,
            "bq": bq, "bk": bk, "bv": bv_s, "bo": bo, "bgo": bgo,
            "gam": gam, "bet": bet,
        })

    last_exc = None
    for _attempt in range(3):
        try:
            res = run_bass_kernel_spmd(nc, in_maps, core_ids=list(range(8)),
                                       trace=_TRACE[0])
            break
        except Exception as exc:  # flaky NRT_EXEC_UNIT errors: retry
            last_exc = exc
            import time
            time.sleep(2.0)
    else:
        raise last_exc
    _LAST_RESULT[0] = res

    y = np.empty((B, S, DM), dtype=np.float32)
    for c in range(8):
        b, blk = divmod(c, 4)
        r0 = blk * SQ
        y[b, r0:r0 + SQ] = res.results[c]["y"].T
    return y


# revision 10
# speedup vs baseline: 1.4752x; 1.4752x over previous
"""Trainium2 Bass kernel for EnhancedMultiHeadAttention (B=2, S=2048, DM=1024, H=16).

Sharding v3: 8 NeuronCores = 2 batches x 4 query-row blocks of 512 rows.
Each core projects K/V only for its OWN 512 rows, then ONE bf16 AllGather
over the 4 cores of each batch rebuilds the full K^T / V in DRAM (K and V
are packed into a single bounce buffer; one collective halves the ~30-60us
per-op firmware cost). A tiny AllGather at t~0 absorbs cross-core launch
skew in parallel with the projection phase, and a tunable stretch of pad
matmuls bridges the residual gather wait so the PE never goes idle long
enough for the HAM clock manager to halve the clock (observed: a ~70us PE
gap locks the clock at k=4 for the rest of the kernel).

The output and gate projections are folded into the attention loop: the
host precomputes Wgo = Wg @ Wo and bgo = Wg @ bo + bg, and as each head
pair's context finishes, 8+8 matmuls accumulate its contribution to
out^T and gate_z^T in SBUF (adds on the otherwise-idle GpSimd engine).
The epilogue is elementwise-only in transposed [DM, 512] layout (sigmoid,
gate mix, layernorm with cross-partition stats via a ones-matmul), and
y^T is transposed on the host.

Precision: projections fp32r; scores q/k in bf16 (still well within the
2e-2 budget); attn @ v in bf16. softmax(attention_weights) folded into
Wv/bv on the host.
"""
import math
import os
import sys

import numpy as np

for _p in ("/opt/trn_rl_repo", "/opt/pypackages"):
    if _p not in sys.path:
        sys.path.append(_p)

import concourse.bass as bass
import concourse.mybir as mybir
import concourse.tile as tile
from concourse import bacc
from concourse.bass_utils import run_bass_kernel_spmd

F32R = mybir.dt.float32r
F32 = mybir.dt.float32
BF16 = mybir.dt.bfloat16
AF = mybir.ActivationFunctionType
ALU = mybir.AluOpType

B, S, DM, H = 2, 2048, 1024, 16
HD = DM // H                  # 64
SQ = 512                      # query rows per core
NP = 128                      # partitions
KC = DM // NP                 # 8 contraction chunks
NT = S // NP                  # 16 key/value tiles
NPAIR = H // 2                # 8 head pairs
NSLOT = 4                     # gather slots (cores per batch)
N512 = 512
SCALE = 1.0 / math.sqrt(HD)
EPS = 1e-5
GROUPS = [[0, 1, 2, 3], [4, 5, 6, 7]]
N_PAD = 96                    # pad matmuls bridging the gather wait

_CACHE = {}
_TRACE = [False]
_LAST_RESULT = [None]


def _bcast(ap_1d, p=NP):
    return bass.AP(tensor=ap_1d.tensor, offset=ap_1d.offset,
                   ap=[[0, p]] + list(ap_1d.ap))


def _build():
    nc = bacc.Bacc("TRN2", target_bir_lowering=False, debug=False,
                   num_devices=8)

    xT_d = nc.dram_tensor("xT", [DM, SQ], F32R, kind="ExternalInput").ap()
    wkT_d = nc.dram_tensor("wkT", [DM, DM], F32R, kind="ExternalInput").ap()
    wvT_d = nc.dram_tensor("wvT", [DM, DM], F32R, kind="ExternalInput").ap()
    wqT_d = nc.dram_tensor("wqT", [DM, DM], F32R, kind="ExternalInput").ap()
    woT_d = nc.dram_tensor("woT", [DM, DM], F32R, kind="ExternalInput").ap()
    wgoT_d = nc.dram_tensor("wgoT", [DM, DM], F32R, kind="ExternalInput").ap()
    bq_d = nc.dram_tensor("bq", [DM], F32, kind="ExternalInput").ap()
    bk_d = nc.dram_tensor("bk", [DM], F32, kind="ExternalInput").ap()
    bv_d = nc.dram_tensor("bv", [DM], F32, kind="ExternalInput").ap()
    bo_d = nc.dram_tensor("bo", [DM], F32, kind="ExternalInput").ap()
    bgo_d = nc.dram_tensor("bgo", [DM], F32, kind="ExternalInput").ap()
    gam_d = nc.dram_tensor("gam", [DM], F32, kind="ExternalInput").ap()
    bet_d = nc.dram_tensor("bet", [DM], F32, kind="ExternalInput").ap()
    y_d = nc.dram_tensor("y", [DM, SQ], F32, kind="ExternalOutput").ap()

    xT_v = xT_d.rearrange("(c p) s -> p c s", p=NP)
    wk_v = wkT_d.rearrange("(c p) d -> p c d", p=NP)
    wv_v = wvT_d.rearrange("(c p) d -> p c d", p=NP)
    wq_v = wqT_d.rearrange("(c p) d -> p c d", p=NP)
    wo_v = woT_d.rearrange("(c p) d -> p c d", p=NP)
    wgo_v = wgoT_d.rearrange("(c p) d -> p c d", p=NP)
    y_v = y_d.rearrange("(c p) s -> p c s", p=NP)

    with tile.TileContext(nc) as tc:
        with tc.tile_pool(name="pers", bufs=1) as pers, \
             tc.tile_pool(name="acc", bufs=1) as acc, \
             tc.tile_pool(name="dram", bufs=1, space="DRAM") as dram:
            # launch-skew absorber: tiny gather issued before any compute
            bar_sb = pers.tile([NP, NSLOT], F32)
            bar_in_dr = dram.tile([NP, NSLOT], F32)
            bar_out_dr = dram.tile([NSLOT, NP, NSLOT], F32)
            nc.vector.memset(bar_sb, 1.0)
            nc.gpsimd.dma_start(out=bar_in_dr[:], in_=bar_sb[:])
            nc.gpsimd.collective_compute(
                "AllGather", mybir.AluOpType.bypass,
                replica_groups=GROUPS,
                ins=[bar_in_dr.opt()], outs=[bar_out_dr.opt()])
            bar_rd = pers.tile([NSLOT, NP, NSLOT], F32)
            nc.gpsimd.dma_start(out=bar_rd, in_=bar_out_dr[:])

            # own x^T chunks: first DMAs in flight
            xT_sb = acc.tile([NP, KC, SQ], F32R)      # own x^T (also residual)
            for kc in range(KC):
                nc.sync.dma_start(out=xT_sb[:, kc, :], in_=xT_v[:, kc, :])

            # per-partition bias/affine scalars ([128, 8]: chunk c, part p
            # maps to feature c*128+p)
            bq_sb = pers.tile([NP, KC], F32)
            bk_sb = pers.tile([NP, KC], F32)
            bo_sb = pers.tile([NP, KC], F32)
            bgo_sb = pers.tile([NP, KC], F32)
            gam_sb = pers.tile([NP, KC], F32)
            bet_sb = pers.tile([NP, KC], F32)
            nc.sync.dma_start(out=bq_sb, in_=bq_d.rearrange("(c p) -> p c", p=NP))
            nc.sync.dma_start(out=bk_sb, in_=bk_d.rearrange("(c p) -> p c", p=NP))
            nc.sync.dma_start(out=bo_sb, in_=bo_d.rearrange("(c p) -> p c", p=NP))
            nc.sync.dma_start(out=bgo_sb, in_=bgo_d.rearrange("(c p) -> p c", p=NP))
            nc.sync.dma_start(out=gam_sb, in_=gam_d.rearrange("(c p) -> p c", p=NP))
            nc.sync.dma_start(out=bet_sb, in_=bet_d.rearrange("(c p) -> p c", p=NP))
            bv_bc = pers.tile([NP, DM], F32)
            nc.sync.dma_start(out=bv_bc, in_=_bcast(bv_d))
            eps_sb = pers.tile([NP, 1], F32)
            nc.vector.memset(eps_sb, EPS)
            ones_f32 = pers.tile([NP, NP], F32)
            nc.vector.memset(ones_f32, 1.0)
            ones_sb = pers.tile([NP, NP], F32R)
            nc.gpsimd.dma_start(out=ones_sb, in_=ones_f32)

            outT_acc = acc.tile([NP, KC, SQ], F32R)   # out^T accumulator
            gateT_acc = acc.tile([NP, KC, SQ], F32R)  # gate logits^T accumulator

            # combined K+V bounce buffers (bf16): cols 0:8 = K^T shard
            # (dt, t), cols 8:16 = V shard (flat (tt, d) as (2tt+h, 512))
            cin_dr = dram.tile([NP, 2 * KC, SQ], BF16)
            cout_dr = dram.tile([NSLOT, NP, 2 * KC, SQ], BF16)

            with tc.tile_pool(name="qres", bufs=1) as qres, \
                 tc.tile_pool(name="kvsh", bufs=1) as kvsh, \
                 tc.tile_pool(name="vqp", bufs=1) as vqp, \
                 tc.tile_pool(name="wsl", bufs=2) as wsl, \
                 tc.tile_pool(name="wop", bufs=4) as wop, \
                 tc.tile_pool(name="kpp", bufs=2) as kpp, \
                 tc.tile_pool(name="ctxp", bufs=2) as ctxp, \
                 tc.tile_pool(name="attp", bufs=3) as attp, \
                 tc.tile_pool(name="pp", bufs=2, space="PSUM") as pp, \
                 tc.tile_pool(name="scop", bufs=2, space="PSUM") as scop, \
                 tc.tile_pool(name="cxp", bufs=2, space="PSUM") as cxp:
                qT_sb = qres.tile([NP, KC, SQ], BF16)
                ksh_sb = kvsh.tile([NP, KC, SQ], BF16)      # own K^T shard
                vsh_sb = kvsh.tile([NP, KC, SQ], BF16)      # own V shard
                vq_sb = vqp.tile([NP, 2 * NT, SQ], BF16)    # gathered V

                # ---- phase 1: own-shard projections + collective ---------
                # K^T shard: ksh[d, t] for all d, own t
                for dt in range(KC):
                    wk_sl = wsl.tile([NP, KC, NP], F32R, tag="w", name="wk_sl")
                    nc.sync.dma_start(out=wk_sl,
                                      in_=wk_v[:, :, dt * NP:(dt + 1) * NP])
                    ps_t = pp.tile([NP, SQ], F32, tag="pj", name="ps_k")
                    for kc in range(KC):
                        nc.tensor.matmul(ps_t, wk_sl[:, kc, :], xT_sb[:, kc, :],
                                         start=(kc == 0), stop=(kc == KC - 1))
                    nc.vector.tensor_scalar_add(ksh_sb[:, dt, :], ps_t,
                                                bk_sb[:, dt:dt + 1])
                nc.sync.dma_start(out=cin_dr[:, 0:KC, :], in_=ksh_sb[:])

                # V shard: vsh[t, d] for own t, all d (bias + head_w folded)
                for h in range(2):
                    wv_sl = wsl.tile([NP, KC, N512], F32R, tag="wv", name="wv_sl")
                    nc.sync.dma_start(out=wv_sl,
                                      in_=wv_v[:, :, h * N512:(h + 1) * N512])
                    for tt in range(NSLOT):
                        ps_t = pp.tile([NP, N512], F32, tag="pj", name="ps_v")
                        for kc in range(KC):
                            nc.tensor.matmul(
                                ps_t, xT_sb[:, kc, tt * NP:(tt + 1) * NP],
                                wv_sl[:, kc, :],
                                start=(kc == 0), stop=(kc == KC - 1))
                        nc.vector.tensor_add(
                            vsh_sb[:, 2 * tt + h, :], ps_t,
                            bv_bc[:, h * N512:(h + 1) * N512])
                nc.sync.dma_start(out=cin_dr[:, KC:2 * KC, :], in_=vsh_sb[:])
                nc.gpsimd.collective_compute(
                    "AllGather", mybir.AluOpType.bypass,
                    replica_groups=GROUPS,
                    ins=[cin_dr.opt()], outs=[cout_dr.opt()])

                # Q^T: [DM, 512] in bf16 (scores run bf16 x bf16)
                for dt in range(KC):
                    wq_sl = wsl.tile([NP, KC, NP], F32R, tag="w", name="wq_sl")
                    nc.sync.dma_start(out=wq_sl,
                                      in_=wq_v[:, :, dt * NP:(dt + 1) * NP])
                    ps_q = pp.tile([NP, SQ], F32, tag="pj", name="ps_q")
                    for kc in range(KC):
                        nc.tensor.matmul(ps_q, wq_sl[:, kc, :], xT_sb[:, kc, :],
                                         start=(kc == 0), stop=(kc == KC - 1))
                    nc.vector.tensor_scalar_add(qT_sb[:, dt, :], ps_q,
                                                bq_sb[:, dt:dt + 1])

                # gathered V -> SBUF
                for r in range(NSLOT):
                    nc.gpsimd.dma_start(
                        out=vq_sb[:, r * KC:(r + 1) * KC, :],
                        in_=cout_dr[r, :, KC:2 * KC, :])

                # pad matmuls: keep the PE (and its HAM clock) busy while
                # the gather lands; the accumulated result is discarded
                # (one PSUM group + a single eviction keeps the verifier happy)
                ps_pad = pp.tile([NP, SQ], F32, tag="pj", name="ps_pad")
                for i in range(N_PAD):
                    nc.tensor.matmul(ps_pad, ones_sb, xT_sb[:, i % KC, :],
                                     start=(i == 0), stop=(i == N_PAD - 1))
                pad_rd = pers.tile([NP, SQ], F32)
                nc.vector.tensor_copy(pad_rd, ps_pad)

                # ---- main loop: attention + incremental out/gate ---------
                for p in range(NPAIR):
                    kp = kpp.tile([NP, NT * NP], BF16, tag="kp", name="kp")
                    for r in range(NSLOT):
                        nc.sync.dma_start(
                            out=kp[:, r * N512:(r + 1) * N512],
                            in_=cout_dr[r, :, p, :])
                    wo_sl = wop.tile([NP, DM], F32R, tag="wo", name="wo_sl")
                    nc.sync.dma_start(out=wo_sl, in_=wo_v[:, p, :])
                    wgo_sl = wop.tile([NP, DM], F32R, tag="wo", name="wgo_sl")
                    nc.sync.dma_start(out=wgo_sl, in_=wgo_v[:, p, :])

                    ctx_ps = cxp.tile([NP, SQ], F32, tag="cx", name="ctx_ps")
                    vh = 0 if p < 4 else 1
                    c0 = (p % 4) * NP
                    for t in range(NT):
                        sco = scop.tile([NP, 2 * SQ], F32, tag="sc", name="sco")
                        nc.tensor.matmul(sco[:, 0:SQ],
                                         kp[0:64, t * NP:(t + 1) * NP],
                                         qT_sb[0:64, p, :],
                                         start=True, stop=True,
                                         tile_position=(0, 0))
                        nc.tensor.matmul(sco[:, SQ:2 * SQ],
                                         kp[64:128, t * NP:(t + 1) * NP],
                                         qT_sb[64:128, p, :],
                                         start=True, stop=True,
                                         tile_position=(64, 0))
                        att_t = attp.tile([NP, 2 * SQ], BF16, tag="at",
                                          name="att_t")
                        nc.scalar.activation(out=att_t, in_=sco, func=AF.Gelu,
                                             scale=SCALE)
                        nc.tensor.matmul(ctx_ps[0:64, :],
                                         vq_sb[:, 2 * t + vh, c0:c0 + 64],
                                         att_t[:, 0:SQ],
                                         start=(t == 0), stop=(t == NT - 1),
                                         tile_position=(0, 0))
                        nc.tensor.matmul(ctx_ps[64:128, :],
                                         vq_sb[:, 2 * t + vh, c0 + 64:c0 + NP],
                                         att_t[:, SQ:2 * SQ],
                                         start=(t == 0), stop=(t == NT - 1),
                                         tile_position=(0, 64))
                    ctx_sb = ctxp.tile([NP, SQ], F32R, tag="cs", name="ctx_sb")
                    nc.vector.tensor_copy(ctx_sb, ctx_ps)

                    # fold pair p into out^T and gate^T (adds on GpSimd)
                    for dt in range(KC):
                        ps_o = pp.tile([NP, SQ], F32, tag="pj", name="ps_o")
                        nc.tensor.matmul(ps_o, wo_sl[:, dt * NP:(dt + 1) * NP],
                                         ctx_sb, start=True, stop=True)
                        if p == 0:
                            nc.vector.tensor_scalar_add(
                                outT_acc[:, dt, :], ps_o, bo_sb[:, dt:dt + 1])
                        else:
                            nc.vector.tensor_add(
                                outT_acc[:, dt, :], ps_o, outT_acc[:, dt, :])
                        ps_g = pp.tile([NP, SQ], F32, tag="pj", name="ps_g")
                        nc.tensor.matmul(ps_g, wgo_sl[:, dt * NP:(dt + 1) * NP],
                                         ctx_sb, start=True, stop=True)
                        if p == 0:
                            nc.vector.tensor_scalar_add(
                                gateT_acc[:, dt, :], ps_g, bgo_sb[:, dt:dt + 1])
                        else:
                            nc.vector.tensor_add(
                                gateT_acc[:, dt, :], ps_g, gateT_acc[:, dt, :])

            # ------------- epilogue: all in transposed [DM, 512] ----------
            with tc.tile_pool(name="ep", bufs=2) as ep, \
                 tc.tile_pool(name="epo", bufs=2) as epo, \
                 tc.tile_pool(name="lns", bufs=1) as lns, \
                 tc.tile_pool(name="spp", bufs=2, space="PSUM") as spp:
                # gate = sigmoid(z)  (in place)
                for g in range(4):
                    nc.scalar.activation(out=gateT_acc[:, 2 * g:2 * g + 2, :],
                                         in_=gateT_acc[:, 2 * g:2 * g + 2, :],
                                         func=AF.Sigmoid)
                musum_ps = spp.tile([NP, SQ], F32, tag="ms", name="musum")
                sqsum_ps = spp.tile([NP, SQ], F32, tag="ms", name="sqsum")
                for dt in range(KC):
                    eng = nc.vector
                    # ypre = gate*(out - x) + 2x  (overwrites outT_acc)
                    t1 = ep.tile([NP, SQ], F32, tag="t1", name="t1")
                    eng.tensor_sub(t1, outT_acc[:, dt, :], xT_sb[:, dt, :])
                    eng.tensor_mul(t1, t1, gateT_acc[:, dt, :])
                    eng.scalar_tensor_tensor(
                        out=outT_acc[:, dt, :], in0=xT_sb[:, dt, :], scalar=2.0,
                        in1=t1, op0=ALU.mult, op1=ALU.add)
                    sq = ep.tile([NP, SQ], F32R, tag="sq", name="sq")
                    eng.tensor_mul(sq, outT_acc[:, dt, :], outT_acc[:, dt, :])
                    nc.tensor.matmul(musum_ps, ones_sb, outT_acc[:, dt, :],
                                     start=(dt == 0), stop=(dt == KC - 1))
                    nc.tensor.matmul(sqsum_ps, ones_sb, sq,
                                     start=(dt == 0), stop=(dt == KC - 1))
                mu_sb = lns.tile([NP, SQ], F32)
                m2_sb = lns.tile([NP, SQ], F32)
                var_sb = lns.tile([NP, SQ], F32)
                rstd_sb = lns.tile([NP, SQ], F32)
                nc.vector.tensor_scalar_mul(mu_sb, musum_ps, 1.0 / DM)
                nc.vector.tensor_scalar_mul(m2_sb, sqsum_ps, 1.0 / DM)
                nc.vector.tensor_mul(var_sb, mu_sb, mu_sb)
                nc.vector.tensor_sub(var_sb, m2_sb, var_sb)
                nc.scalar.activation(out=var_sb, in_=var_sb, func=AF.Sqrt,
                                     bias=eps_sb)
                nc.vector.reciprocal(rstd_sb, var_sb)
                for dt in range(KC):
                    eng = nc.vector
                    yt = epo.tile([NP, SQ], F32, tag="y", name="yt")
                    eng.tensor_sub(yt, outT_acc[:, dt, :], mu_sb)
                    eng.tensor_mul(yt, yt, rstd_sb)
                    eng.tensor_scalar(
                        out=yt, in0=yt,
                        scalar1=gam_sb[:, dt:dt + 1],
                        scalar2=bet_sb[:, dt:dt + 1],
                        op0=ALU.mult, op1=ALU.add)
                    nc.sync.dma_start(out=y_v[:, dt, :], in_=yt)

    nc.compile()
    return nc


def kernel(x, Wq, bq, Wk, bk, Wv, bv, Wo, bo, Wg, bg, attention_weights,
           ln_gamma, ln_beta):
    x = np.asarray(x, dtype=np.float32)
    f32 = lambda a: np.ascontiguousarray(np.asarray(a, dtype=np.float32))
    Wq, Wk, Wv, Wo, Wg = map(f32, (Wq, Wk, Wv, Wo, Wg))
    bq, bk, bv, bo, bg = map(f32, (bq, bk, bv, bo, bg))
    aw, gam, bet = map(f32, (attention_weights, ln_gamma, ln_beta))

    if "nc" not in _CACHE:
        _CACHE["nc"] = _build()
    nc = _CACHE["nc"]

    # fold softmax(attention_weights) into Wv / bv
    e = np.exp(aw - aw.max())
    head_w = (e / e.sum()).astype(np.float32)
    hw_exp = np.repeat(head_w, HD)              # [DM]
    Wv_s = Wv * hw_exp[:, None]
    bv_s = bv * hw_exp

    # fold gate projection through the output projection
    Wgo = (Wg.astype(np.float64) @ Wo.astype(np.float64)).astype(np.float32)
    bgo = (Wg.astype(np.float64) @ bo.astype(np.float64)).astype(np.float32) + bg

    wqT = np.ascontiguousarray(Wq.T)
    wkT = np.ascontiguousarray(Wk.T)
    wvT = np.ascontiguousarray(Wv_s.T)
    woT = np.ascontiguousarray(Wo.T)
    wgoT = np.ascontiguousarray(Wgo.T)

    in_maps = []
    for c in range(8):
        b, blk = divmod(c, 4)
        r0 = blk * SQ
        in_maps.append({
            "xT": np.ascontiguousarray(x[b, r0:r0 + SQ].T),
            "wkT": wkT, "wvT": wvT, "wqT": wqT, "woT": woT, "wgoT": wgoT,
            "bq": bq, "bk": bk, "bv": bv_s, "bo": bo, "bgo": bgo,
            "gam": gam, "bet": bet,
        })

    last_exc = None
    for _attempt in range(3):
        try:
            res = run_bass_kernel_spmd(nc, in_maps, core_ids=list(range(8)),
                                       trace=_TRACE[0])
            break
        except Exception as exc:  # flaky NRT_EXEC_UNIT errors: retry
            last_exc = exc
            import time
            time.sleep(2.0)
    else:
        raise last_exc
    _LAST_RESULT[0] = res

    y = np.empty((B, S, DM), dtype=np.float32)
    for c in range(8):
        b, blk = divmod(c, 4)
        r0 = blk * SQ
        y[b, r0:r0 + SQ] = res.results[c]["y"].T
    return y
